# revision 38
# baseline (speedup 1.0000x reference)
"""BioWaveKAN fused kernel for 8 Trainium2 NeuronCores.

Host feeds u = (x - translate)/clamp(scale) directly (the scale folds into the
base weights; the translate term is a per-column constant that BatchNorm's
mean-subtraction cancels exactly). Device computes

  y = wavelet(u) @ (pi^-1/4 * Ww).T + u @ (0.3 * diag(s) Wb).T
  out = BatchNorm1d(y)   (training-mode batch stats, all-reduced across cores)

Sharding: data-parallel over batch (8 x 512 rows).

Single-pass contraction (K=4096 accumulated in PSUM over 32 k-tiles) per
output group; o-tiles split into 5 groups [4,4,4,3,1]. BN stats ride three
AllReduces (g0+g1, g2, g3+g4); a dummy warm-up AllReduce at kernel start
absorbs the ~60us first-collective ncfw cost under the matmul stream, and
the g3+g4 split keeps only one o-tile's drain after the last matmul.
Matmul order [g0u, g1u, g0w, g1w, g2, g3, g4] gives the on-device wavelet
(sin via exact magic-number range reduction, exp; quad-tile [128,2048] ops)
time to finish before the first wavelet k-tile is consumed. Early input DMAs
split across the sync and scalar HWDGE queues (cold-start DMA is the
matmul-start bottleneck); stats-in DMAs ride the scalar queue behind the
drain squares so collective triggers never wait on the weight stream.
"""
import math

import numpy as np

from concourse import bacc
import concourse.tile as tile
import concourse.mybir as mybir
from concourse.bass_utils import run_bass_kernel_spmd

F32 = mybir.dt.float32
F16 = mybir.dt.float16
AF = mybir.ActivationFunctionType
OP = mybir.AluOpType

B = 4096          # batch
D = 2048          # in_dim == out_dim
NCORES = 8
BS = B // NCORES  # batch shard per core (512)
NIT = D // 128    # i-tiles (16)
NKT = 2 * NIT     # contraction tiles (32): 0..15 = u, 16..31 = wavelet
NOT = D // 128    # o-tiles (16)
BN_EPS = 1e-5
TWO_PI = 2.0 * math.pi
MAGIC = 1.5 * 2.0 ** 23

# o-tile groups: last groups small so the tail AllReduce is tiny
GROUPS = [(0, 4), (4, 4), (8, 4), (12, 3), (15, 1)]  # (first o-tile, count)
NG = len(GROUPS)

_CACHE = {}


def _build_nc():
    nc = bacc.Bacc()

    uT_d = nc.dram_tensor("uT", (D, BS), F16, kind="ExternalInput")
    wT_d = nc.dram_tensor("wT", (2 * D, D), F16, kind="ExternalInput")
    cst_d = nc.dram_tensor("cst", (128, 2 * NOT), F32, kind="ExternalInput")

    yT_d = nc.dram_tensor("yT", (D, BS), F16, kind="ExternalOutput")

    uT_t = uT_d[:].rearrange("(kt p) b -> p kt b", p=128)      # [128, 16, BS]
    wT_t = wT_d[:].rearrange("(kt p) o -> p kt o", p=128)      # [128, 32, D]
    yT_t = yT_d[:].rearrange("(mt p) b -> p mt b", p=128)      # [128, 16, BS]

    # stats column layout per group: (sum base col, sumsq base col).
    # Groups 3+4 share one [sums(4) | sqs(4)] block at cols 24:32 so their
    # finalize after the last AllReduce is a single 4-col pass.
    SCOLS = {0: (0, 4), 1: (8, 12), 2: (16, 20), 3: (24, 28), 4: (27, 31)}

    with tile.TileContext(nc) as tc:
        with (
            tc.tile_pool(name="big", bufs=1) as big,
            tc.tile_pool(name="small", bufs=1) as small,
            tc.tile_pool(name="wp", bufs=5) as wp,
            tc.tile_pool(name="yp", bufs=3) as yp,
            tc.tile_pool(name="ytp", bufs=3) as ytp,
            tc.tile_pool(name="scr", bufs=2) as scrp,
            tc.tile_pool(name="ps", bufs=8, space="PSUM") as ps,
            tc.tile_pool(name="dram", bufs=1, space="DRAM") as dram,
        ):
            rhs = big.tile([128, NKT, BS], F16)

            # ---- warm-up collective: the first AllReduce of an execution
            # pays a large one-time ncfw/descriptor cost; burn it under the
            # matmul stream with a dummy 1KB AllReduce issued immediately ----
            wz = small.tile([128, 2], F32)
            nc.vector.memset(wz[:], 0.0)
            ibw = dram.tile([128, 2], F32, name="ibw")
            obw = dram.tile([128, 2], F32, name="obw")
            nc.sync.dma_start(ibw[:], wz[:])
            nc.gpsimd.collective_compute(
                "AllReduce", OP.add,
                replica_groups=[list(range(NCORES))],
                ins=[ibw.opt()], outs=[obw.opt()])

            # ---- input DMAs: interleave u quads with the matching weight
            #      chunks so the matmul stream never outruns the (slow, cold)
            #      early DMA bandwidth; cst (slow small-descriptor transfer)
            #      goes late, it is not needed until the first finalize ----
            # first weight chunks ride the scalar queue so they transfer
            # concurrently with the u chunks on the sync queue (the cold
            # early DMA window is bandwidth/latency limited)
            # first chunks ride the scalar and vector queues: their engine
            # preambles finish before the sync queue's framework table DMAs,
            # so the first matmul's operands land ~2-3us earlier
            wg0u = wp.tile([128, NIT, 512], F16, tag="w", name="wg0u")
            nc.scalar.dma_start(wg0u[:, 0:2, :], wT_t[:, 0:2, 0:512])
            nc.scalar.dma_start(wg0u[:, 2:4, :], wT_t[:, 2:4, 0:512])
            nc.gpsimd.dma_start(rhs[:, 0:2, :], uT_t[:, 0:2, :])
            nc.gpsimd.dma_start(rhs[:, 2:4, :], uT_t[:, 2:4, :])
            for c in range(1, 4):
                nc.sync.dma_start(rhs[:, 4 * c:4 * c + 4, :],
                                  uT_t[:, 4 * c:4 * c + 4, :])
                nc.scalar.dma_start(wg0u[:, 4 * c:4 * c + 4, :],
                                    wT_t[:, 4 * c:4 * c + 4, 0:512])

            wg1u = wp.tile([128, NIT, 512], F16, tag="w", name="wg1u")
            nc.sync.dma_start(wg1u[:], wT_t[:, 0:NIT, 512:1024])
            wg0w = wp.tile([128, NIT, 512], F16, tag="w", name="wg0w")
            nc.sync.dma_start(wg0w[:], wT_t[:, NIT:NKT, 0:512])
            wg1w = wp.tile([128, NIT, 512], F16, tag="w", name="wg1w")
            nc.sync.dma_start(wg1w[:], wT_t[:, NIT:NKT, 512:1024])

            wtiles = {(0, 0): wg0u, (0, 1): wg0w,
                      (1, 0): wg1u, (1, 1): wg1w}
            cstt = small.tile([128, 2 * NOT], F32)
            for g in range(2, NG):
                m0, nt = GROUPS[g]
                oc0, ocw = m0 * 128, nt * 128
                for h in range(2):
                    wt = wp.tile([128, NIT, ocw], F16, tag="w",
                                 name=f"w_g{g}_{h}")
                    nc.sync.dma_start(
                        wt[:], wT_t[:, h * NIT:(h + 1) * NIT, oc0:oc0 + ocw])
                    wtiles[(g, h)] = wt
                if g == 2:
                    nc.sync.dma_start(cstt[:], cst_d[:])
            gmt = cstt[:, 0:NOT]
            btt = cstt[:, NOT:2 * NOT]

            magict = small.tile([128, 1], F32)
            nc.vector.memset(magict[:], MAGIC)
            zbt = small.tile([128, 1], F32)
            nc.vector.memset(zbt[:], 0.0)
            epst = small.tile([128, 1], F32)
            nc.vector.memset(epst[:], BN_EPS)

            # ---- wavelet: quad tiles [128, 2048] covering 4 k-tiles ----
            # cos(3u) = sin(2*pi*t), t = u*(3/2pi) + 1/4; r = t - round(t)
            C1 = 3.0 / TWO_PI
            tq, kq = {}, {}
            for q in range(4):
                uq = rhs[:, 4 * q:4 * q + 4, :]
                tt = scrp.tile([128, 4, BS], F32, tag="t", name=f"t_{q}")
                nc.vector.tensor_scalar(out=tt[:], in0=uq,
                                        scalar1=C1, scalar2=0.25,
                                        op0=OP.mult, op1=OP.add)
                kt_ = scrp.tile([128, 4, BS], F32, tag="k", name=f"k_{q}")
                nc.vector.tensor_scalar(out=kt_[:], in0=tt[:],
                                        scalar1=magict[:], scalar2=magict[:],
                                        op0=OP.add, op1=OP.subtract)
                # r = t - k, in place into t
                nc.vector.tensor_tensor(tt[:], tt[:], kt_[:], op=OP.subtract)
                tq[q] = tt

            # ACT: sins (trig set), then exp-set switch, then square/exp
            for q in range(4):
                nc.scalar.activation(rhs[:, NIT + 4 * q:NIT + 4 * q + 4, :],
                                     tq[q][:], AF.Sin,
                                     bias=zbt[:], scale=TWO_PI)
            eq = {}
            for q in range(4):
                uq = rhs[:, 4 * q:4 * q + 4, :]
                sq = scrp.tile([128, 4, BS], F16, tag="sq", name=f"sq_{q}")
                nc.scalar.activation(sq[:], uq, AF.Square, bias=zbt[:])
                et = scrp.tile([128, 4, BS], F16, tag="e", name=f"e_{q}")
                nc.scalar.activation(et[:], sq[:], AF.Exp,
                                     bias=zbt[:], scale=-0.5)
                eq[q] = et
            # wavelet = sin * exp, in place in rhs (fp16, 2x DVE mode)
            for q in range(4):
                wsl = rhs[:, NIT + 4 * q:NIT + 4 * q + 4, :]
                nc.vector.tensor_tensor(wsl, wsl, eq[q][:], op=OP.mult)

            # prefetch the sqrt table set (Square lives in it too, so later
            # drain squares don't reload; the finalize Sqrts are then free)
            sqpre = small.tile([128, 1], F32)
            nc.scalar.activation(sqpre[:], zbt[:], AF.Sqrt, bias=epst[:])

            # ---- matmuls: single-pass K=4096 per group, interleaved so
            #      g0/g1's wavelet half runs after g1's u half ----
            stats = small.tile([128, 2 * NOT], F32)
            red = small.tile([128, 2 * NOT], F32)
            ab = small.tile([128, 2 * NOT], F32)  # a cols 0..15, b cols 16..31

            psums = {}
            for g in range(NG):
                m0, nt = GROUPS[g]
                for ml in range(nt):
                    psums[(g, ml)] = ps.tile([128, BS], F32, tag="ps",
                                             name=f"ps_{g}_{ml}")

            ytiles, yttiles = {}, {}
            for g in range(NG):
                m0, nt = GROUPS[g]
                ytiles[g] = yp.tile([128, nt, BS], F32, tag="y",
                                    name=f"y_g{g}")
                yttiles[g] = ytp.tile([128, nt, BS], F16, tag="yt",
                                      name=f"yt_g{g}")

            # Stats exchange blocks: (stats col range) A=g0+g1, B=g2, C=g3+g4
            ARB = {"a": (0, 16), "b": (16, 24), "c": (24, 32)}
            ibs, obs = {}, {}
            for k, (c0, c1) in ARB.items():
                w = c1 - c0
                ibs[k] = dram.tile([128, w], F32, name=f"ib{k}")
                obs[k] = dram.tile([128, w], F32, name=f"ob{k}")

            def mm_phase(g, h):
                m0, nt = GROUPS[g]
                wt = wtiles[(g, h)]
                for kt in range(NIT):
                    wsl = wt[:, kt, :]
                    for ml in range(nt):
                        nc.tensor.matmul(
                            psums[(g, ml)][:],
                            wsl[:, ml * 128:(ml + 1) * 128],
                            rhs[:, h * NIT + kt, :],
                            start=(h == 0 and kt == 0),
                            stop=(h == 1 and kt == NIT - 1))

            def drain(g):
                m0, nt = GROUPS[g]
                sc, qc = SCOLS[g]
                for ml in range(nt):
                    # y to SBUF + sum(y) on DVE; sum(y^2) on ACT from PSUM
                    nc.vector.tensor_scalar(
                        out=ytiles[g][:, ml, :], in0=psums[(g, ml)][:],
                        scalar1=1.0, scalar2=0.0,
                        op0=OP.mult, op1=OP.add,
                        accum_out=stats[:, sc + ml:sc + ml + 1])
                    nc.scalar.activation(
                        yttiles[g][:, ml, :], psums[(g, ml)][:], AF.Square,
                        bias=zbt[:],
                        accum_out=stats[:, qc + ml:qc + ml + 1])

            def allreduce(k):
                # stats-in DMA rides the scalar queue right behind the
                # drain squares so it never blocks (or is blocked by) the
                # sync queue's weight/store stream
                c0, c1 = ARB[k]
                nc.scalar.dma_start(ibs[k][:], stats[:, c0:c1])
                nc.gpsimd.collective_compute(
                    "AllReduce", OP.add,
                    replica_groups=[list(range(NCORES))],
                    ins=[ibs[k].opt()], outs=[obs[k].opt()])

            def fetch_red(k):
                c0, c1 = ARB[k]
                nc.sync.dma_start(red[:, c0:c1], obs[k][:])

            def finalize(m0, nt, sc, qc):
                sums = red[:, sc:sc + nt]
                sqs = red[:, qc:qc + nt]
                mean = small.tile([128, nt], F32, name=f"mean{m0}")
                nc.vector.tensor_single_scalar(
                    out=mean[:], in_=sums, scalar=1.0 / B, op=OP.mult)
                msq = small.tile([128, nt], F32, name=f"msq{m0}")
                nc.vector.tensor_single_scalar(
                    out=msq[:], in_=sqs, scalar=1.0 / B, op=OP.mult)
                var = small.tile([128, nt], F32, name=f"var{m0}")
                nc.vector.tensor_tensor(var[:], mean[:], mean[:], op=OP.mult)
                nc.vector.tensor_tensor(var[:], msq[:], var[:],
                                        op=OP.subtract)
                stdt = small.tile([128, nt], F32, name=f"std{m0}")
                nc.scalar.activation(stdt[:], var[:], AF.Sqrt, bias=epst[:])
                rstd = small.tile([128, nt], F32, name=f"rstd{m0}")
                nc.vector.reciprocal(out=rstd[:], in_=stdt[:])
                acols = ab[:, m0:m0 + nt]
                bcols = ab[:, NOT + m0:NOT + m0 + nt]
                nc.vector.tensor_tensor(acols, gmt[:, m0:m0 + nt], rstd[:],
                                        op=OP.mult)
                nc.vector.tensor_tensor(bcols, mean[:], acols, op=OP.mult)
                nc.vector.tensor_tensor(bcols, btt[:, m0:m0 + nt], bcols,
                                        op=OP.subtract)

            def normalize_store(g, split=False):
                # split=True: put the last tile's affine on ACT (Identity with
                # per-partition scale/bias) so the post-AllReduce tail chain
                # runs on two engines
                m0, nt = GROUPS[g]
                for ml in range(nt):
                    m = m0 + ml
                    if split and ml == nt - 1:
                        nc.scalar.activation(
                            yttiles[g][:, ml, :], ytiles[g][:, ml, :],
                            AF.Identity, bias=ab[:, NOT + m:NOT + m + 1],
                            scale=ab[:, m:m + 1])
                    else:
                        nc.vector.tensor_scalar(
                            out=yttiles[g][:, ml, :], in0=ytiles[g][:, ml, :],
                            scalar1=ab[:, m:m + 1],
                            scalar2=ab[:, NOT + m:NOT + m + 1],
                            op0=OP.mult, op1=OP.add)
                nc.sync.dma_start(yT_t[:, m0:m0 + nt, :], yttiles[g][:])

            # MM stream: g0u, g1u, g0w, g1w then g2, g3, g4 full
            mm_phase(0, 0)
            mm_phase(1, 0)
            mm_phase(0, 1)
            drain(0)
            mm_phase(1, 1)
            drain(1)
            allreduce("a")
            mm_phase(2, 0)
            mm_phase(2, 1)
            drain(2)
            allreduce("b")
            fetch_red("a")
            finalize(0, 4, 0, 4)
            finalize(4, 4, 8, 12)
            normalize_store(0)
            normalize_store(1)
            mm_phase(3, 0)
            mm_phase(3, 1)
            drain(3)
            mm_phase(4, 0)
            mm_phase(4, 1)
            drain(4)
            allreduce("c")
            fetch_red("b")
            finalize(8, 4, 16, 20)
            normalize_store(2)
            fetch_red("c")
            finalize(12, 4, 24, 28)
            normalize_store(3, split=True)
            normalize_store(4)

    nc.compile()
    return nc


def _get_nc():
    if "nc" not in _CACHE:
        _CACHE["nc"] = _build_nc()
    return _CACHE["nc"]


def _fold(v):
    """(1, D) or (D,) feature vector -> (128, NOT) column-per-o-tile layout."""
    return np.ascontiguousarray(np.asarray(v).reshape(NOT, 128).T).astype(
        np.float32)


def kernel(x, scale, translate, wave_weight, base_weight, gamma, beta):
    x = np.asarray(x, dtype=np.float32)
    scale = np.asarray(scale, dtype=np.float32).reshape(1, D)
    translate = np.asarray(translate, dtype=np.float32).reshape(1, D)
    wave_weight = np.asarray(wave_weight, dtype=np.float32)
    base_weight = np.asarray(base_weight, dtype=np.float32)
    gamma = np.asarray(gamma, dtype=np.float32).reshape(D)
    beta = np.asarray(beta, dtype=np.float32).reshape(D)

    sc = np.maximum(scale, 1e-3)                              # (1, D)
    u = (x - translate) / sc                                  # (B, D)

    # base_out = x @ Wb.T = u @ (diag(sc) Wb.T) + const_row; the const_row
    # shifts every batch row of a column equally, so BN's mean-subtraction
    # cancels it exactly and it is dropped.
    wbase = 0.3 * sc.reshape(D, 1) * base_weight.T            # (D, D) = [f,o]
    wwave = (math.pi ** -0.25) * wave_weight.T                # (D, D)
    wcat = np.ascontiguousarray(
        np.concatenate([wbase, wwave], axis=0).astype(np.float16))  # (2D, D)

    uT = np.ascontiguousarray(u.T.astype(np.float16))         # (D, B)

    cst = np.concatenate([_fold(gamma), _fold(beta)], axis=1)
    common = dict(wT=wcat, cst=np.ascontiguousarray(cst))
    in_maps = [
        dict(uT=np.ascontiguousarray(uT[:, c * BS:(c + 1) * BS]), **common)
        for c in range(NCORES)
    ]

    nc = _get_nc()
    res = run_bass_kernel_spmd(nc, in_maps, core_ids=list(range(NCORES)),
                               **_CACHE.pop("run_kwargs", {}))
    _CACHE["last_res"] = res
    yT = np.concatenate([res.results[c]["yT"] for c in range(NCORES)], axis=1)
    return np.ascontiguousarray(yT.T.astype(np.float32))


# revision 40
# speedup vs baseline: 1.0435x; 1.0435x over previous
"""BioWaveKAN fused kernel for 8 Trainium2 NeuronCores.

Host feeds u = (x - translate)/clamp(scale) directly (the scale folds into the
base weights; the translate term is a per-column constant that BatchNorm's
mean-subtraction cancels exactly). Device computes

  y = wavelet(u) @ (pi^-1/4 * Ww).T + u @ (0.3 * diag(s) Wb).T
  out = BatchNorm1d(y)   (training-mode batch stats, all-reduced across cores)

Sharding: data-parallel over batch (8 x 512 rows).

Single-pass contraction (K=4096 accumulated in PSUM over 32 k-tiles) per
output group; o-tiles split into 5 groups [4,4,4,3,1]. BN stats ride three
AllReduces (g0+g1, g2, g3+g4); a dummy warm-up AllReduce at kernel start
absorbs the ~60us first-collective ncfw cost under the matmul stream, and
the g3+g4 split keeps only one o-tile's drain after the last matmul.
Matmul order [g0u, g1u, g0w, g1w, g2, g3, g4] gives the on-device wavelet
(sin via exact magic-number range reduction, exp; quad-tile [128,2048] ops)
time to finish before the first wavelet k-tile is consumed. Early input DMAs
split across the sync and scalar HWDGE queues (cold-start DMA is the
matmul-start bottleneck); stats-in DMAs ride the scalar queue behind the
drain squares so collective triggers never wait on the weight stream.
"""
import math

import numpy as np

from concourse import bacc
import concourse.tile as tile
import concourse.mybir as mybir
from concourse.bass_utils import run_bass_kernel_spmd

F32 = mybir.dt.float32
F16 = mybir.dt.float16
AF = mybir.ActivationFunctionType
OP = mybir.AluOpType

B = 4096          # batch
D = 2048          # in_dim == out_dim
NCORES = 8
BS = B // NCORES  # batch shard per core (512)
NIT = D // 128    # i-tiles (16)
NKT = 2 * NIT     # contraction tiles (32): 0..15 = u, 16..31 = wavelet
NOT = D // 128    # o-tiles (16)
BN_EPS = 1e-5
TWO_PI = 2.0 * math.pi
MAGIC = 1.5 * 2.0 ** 23

# o-tile groups: last groups small so the tail AllReduce is tiny
GROUPS = [(0, 4), (4, 4), (8, 4), (12, 3), (15, 1)]  # (first o-tile, count)
NG = len(GROUPS)

_CACHE = {}


def _build_nc():
    nc = bacc.Bacc()

    uT_d = nc.dram_tensor("uT", (D, BS), F16, kind="ExternalInput")
    wT_d = nc.dram_tensor("wT", (2 * D, D), F16, kind="ExternalInput")
    cst_d = nc.dram_tensor("cst", (128, 2 * NOT), F32, kind="ExternalInput")

    yT_d = nc.dram_tensor("yT", (D, BS), F16, kind="ExternalOutput")

    uT_t = uT_d[:].rearrange("(kt p) b -> p kt b", p=128)      # [128, 16, BS]
    wT_t = wT_d[:].rearrange("(kt p) o -> p kt o", p=128)      # [128, 32, D]
    yT_t = yT_d[:].rearrange("(mt p) b -> p mt b", p=128)      # [128, 16, BS]

    # stats column layout per group: (sum base col, sumsq base col).
    # Groups 3+4 share one [sums(4) | sqs(4)] block at cols 24:32 so their
    # finalize after the last AllReduce is a single 4-col pass.
    SCOLS = {0: (0, 4), 1: (8, 12), 2: (16, 20), 3: (24, 28), 4: (27, 31)}

    with tile.TileContext(nc) as tc:
        with (
            tc.tile_pool(name="big", bufs=1) as big,
            tc.tile_pool(name="small", bufs=1) as small,
            tc.tile_pool(name="wp", bufs=5) as wp,
            tc.tile_pool(name="yp", bufs=3) as yp,
            tc.tile_pool(name="ytp", bufs=3) as ytp,
            tc.tile_pool(name="scr", bufs=2) as scrp,
            tc.tile_pool(name="ps", bufs=8, space="PSUM") as ps,
            tc.tile_pool(name="dram", bufs=1, space="DRAM") as dram,
        ):
            rhs = big.tile([128, NKT, BS], F16)

            # ---- warm-up collective: the first AllReduce of an execution
            # pays a large one-time ncfw/descriptor cost; burn it under the
            # matmul stream with a dummy 1KB AllReduce issued immediately ----
            wz = small.tile([128, 2], F32)
            nc.vector.memset(wz[:], 0.0)
            ibw = dram.tile([128, 2], F32, name="ibw")
            obw = dram.tile([128, 2], F32, name="obw")
            nc.sync.dma_start(ibw[:], wz[:])
            nc.gpsimd.collective_compute(
                "AllReduce", OP.add,
                replica_groups=[list(range(NCORES))],
                ins=[ibw.opt()], outs=[obw.opt()])
            # second warm-up: the first post-cold collective still runs
            # slower (~26us) than steady-state (~15us); burn that too
            ibw2 = dram.tile([128, 2], F32, name="ibw2")
            obw2 = dram.tile([128, 2], F32, name="obw2")
            nc.sync.dma_start(ibw2[:], wz[:])
            nc.gpsimd.collective_compute(
                "AllReduce", OP.add,
                replica_groups=[list(range(NCORES))],
                ins=[ibw2.opt()], outs=[obw2.opt()])

            # ---- input DMAs: interleave u quads with the matching weight
            #      chunks so the matmul stream never outruns the (slow, cold)
            #      early DMA bandwidth; cst (slow small-descriptor transfer)
            #      goes late, it is not needed until the first finalize ----
            # first weight chunks ride the scalar queue so they transfer
            # concurrently with the u chunks on the sync queue (the cold
            # early DMA window is bandwidth/latency limited)
            # first chunks ride the scalar and vector queues: their engine
            # preambles finish before the sync queue's framework table DMAs,
            # so the first matmul's operands land ~2-3us earlier
            wg0u = wp.tile([128, NIT, 512], F16, tag="w", name="wg0u")
            nc.scalar.dma_start(wg0u[:, 0:2, :], wT_t[:, 0:2, 0:512])
            nc.scalar.dma_start(wg0u[:, 2:4, :], wT_t[:, 2:4, 0:512])
            nc.sync.dma_start(rhs[:, 0:2, :], uT_t[:, 0:2, :])
            nc.sync.dma_start(rhs[:, 2:4, :], uT_t[:, 2:4, :])
            for c in range(1, 4):
                nc.sync.dma_start(rhs[:, 4 * c:4 * c + 4, :],
                                  uT_t[:, 4 * c:4 * c + 4, :])
                nc.scalar.dma_start(wg0u[:, 4 * c:4 * c + 4, :],
                                    wT_t[:, 4 * c:4 * c + 4, 0:512])

            wg1u = wp.tile([128, NIT, 512], F16, tag="w", name="wg1u")
            nc.sync.dma_start(wg1u[:], wT_t[:, 0:NIT, 512:1024])
            wg0w = wp.tile([128, NIT, 512], F16, tag="w", name="wg0w")
            nc.sync.dma_start(wg0w[:], wT_t[:, NIT:NKT, 0:512])
            wg1w = wp.tile([128, NIT, 512], F16, tag="w", name="wg1w")
            nc.sync.dma_start(wg1w[:], wT_t[:, NIT:NKT, 512:1024])

            wtiles = {(0, 0): wg0u, (0, 1): wg0w,
                      (1, 0): wg1u, (1, 1): wg1w}
            cstt = small.tile([128, 2 * NOT], F32)
            for g in range(2, NG):
                m0, nt = GROUPS[g]
                oc0, ocw = m0 * 128, nt * 128
                for h in range(2):
                    wt = wp.tile([128, NIT, ocw], F16, tag="w",
                                 name=f"w_g{g}_{h}")
                    nc.sync.dma_start(
                        wt[:], wT_t[:, h * NIT:(h + 1) * NIT, oc0:oc0 + ocw])
                    wtiles[(g, h)] = wt
                if g == 2:
                    nc.sync.dma_start(cstt[:], cst_d[:])
            gmt = cstt[:, 0:NOT]
            btt = cstt[:, NOT:2 * NOT]

            magict = small.tile([128, 1], F32)
            nc.vector.memset(magict[:], MAGIC)
            zbt = small.tile([128, 1], F32)
            nc.vector.memset(zbt[:], 0.0)
            epst = small.tile([128, 1], F32)
            nc.vector.memset(epst[:], BN_EPS)

            # ---- wavelet: quad tiles [128, 2048] covering 4 k-tiles ----
            # cos(3u) = sin(2*pi*t), t = u*(3/2pi) + 1/4; r = t - round(t)
            C1 = 3.0 / TWO_PI
            tq, kq = {}, {}
            for q in range(4):
                uq = rhs[:, 4 * q:4 * q + 4, :]
                tt = scrp.tile([128, 4, BS], F32, tag="t", name=f"t_{q}")
                nc.vector.tensor_scalar(out=tt[:], in0=uq,
                                        scalar1=C1, scalar2=0.25,
                                        op0=OP.mult, op1=OP.add)
                kt_ = scrp.tile([128, 4, BS], F32, tag="k", name=f"k_{q}")
                nc.vector.tensor_scalar(out=kt_[:], in0=tt[:],
                                        scalar1=magict[:], scalar2=magict[:],
                                        op0=OP.add, op1=OP.subtract)
                # r = t - k, in place into t
                nc.vector.tensor_tensor(tt[:], tt[:], kt_[:], op=OP.subtract)
                tq[q] = tt

            # ACT: sins (trig set), then exp-set switch, then square/exp
            for q in range(4):
                nc.scalar.activation(rhs[:, NIT + 4 * q:NIT + 4 * q + 4, :],
                                     tq[q][:], AF.Sin,
                                     bias=zbt[:], scale=TWO_PI)
            eq = {}
            for q in range(4):
                uq = rhs[:, 4 * q:4 * q + 4, :]
                sq = scrp.tile([128, 4, BS], F16, tag="sq", name=f"sq_{q}")
                nc.scalar.activation(sq[:], uq, AF.Square, bias=zbt[:])
                et = scrp.tile([128, 4, BS], F16, tag="e", name=f"e_{q}")
                nc.scalar.activation(et[:], sq[:], AF.Exp,
                                     bias=zbt[:], scale=-0.5)
                eq[q] = et
            # wavelet = sin * exp, in place in rhs (fp16, 2x DVE mode)
            for q in range(4):
                wsl = rhs[:, NIT + 4 * q:NIT + 4 * q + 4, :]
                nc.vector.tensor_tensor(wsl, wsl, eq[q][:], op=OP.mult)

            # prefetch the sqrt table set (Square lives in it too, so later
            # drain squares don't reload; the finalize Sqrts are then free)
            sqpre = small.tile([128, 1], F32)
            nc.scalar.activation(sqpre[:], zbt[:], AF.Sqrt, bias=epst[:])

            # ---- matmuls: single-pass K=4096 per group, interleaved so
            #      g0/g1's wavelet half runs after g1's u half ----
            stats = small.tile([128, 2 * NOT], F32)
            red = small.tile([128, 2 * NOT], F32)
            ab = small.tile([128, 2 * NOT], F32)  # a cols 0..15, b cols 16..31

            psums = {}
            for g in range(NG):
                m0, nt = GROUPS[g]
                for ml in range(nt):
                    psums[(g, ml)] = ps.tile([128, BS], F32, tag="ps",
                                             name=f"ps_{g}_{ml}")

            ytiles, yttiles = {}, {}
            for g in range(NG):
                m0, nt = GROUPS[g]
                ytiles[g] = yp.tile([128, nt, BS], F32, tag="y",
                                    name=f"y_g{g}")
                yttiles[g] = ytp.tile([128, nt, BS], F16, tag="yt",
                                      name=f"yt_g{g}")

            # Stats exchange blocks: (stats col range) A=g0+g1, B=g2, C=g3+g4
            ARB = {"a": (0, 16), "b": (16, 24), "c": (24, 32)}
            ibs, obs = {}, {}
            for k, (c0, c1) in ARB.items():
                w = c1 - c0
                ibs[k] = dram.tile([128, w], F32, name=f"ib{k}")
                obs[k] = dram.tile([128, w], F32, name=f"ob{k}")

            def mm_phase(g, h):
                m0, nt = GROUPS[g]
                wt = wtiles[(g, h)]
                for kt in range(NIT):
                    wsl = wt[:, kt, :]
                    for ml in range(nt):
                        nc.tensor.matmul(
                            psums[(g, ml)][:],
                            wsl[:, ml * 128:(ml + 1) * 128],
                            rhs[:, h * NIT + kt, :],
                            start=(h == 0 and kt == 0),
                            stop=(h == 1 and kt == NIT - 1))

            def drain(g):
                m0, nt = GROUPS[g]
                sc, qc = SCOLS[g]
                for ml in range(nt):
                    # y to SBUF + sum(y) on DVE; sum(y^2) on ACT from PSUM
                    nc.vector.tensor_scalar(
                        out=ytiles[g][:, ml, :], in0=psums[(g, ml)][:],
                        scalar1=1.0, scalar2=0.0,
                        op0=OP.mult, op1=OP.add,
                        accum_out=stats[:, sc + ml:sc + ml + 1])
                    nc.scalar.activation(
                        yttiles[g][:, ml, :], psums[(g, ml)][:], AF.Square,
                        bias=zbt[:],
                        accum_out=stats[:, qc + ml:qc + ml + 1])

            def allreduce(k):
                # stats-in DMA rides the scalar queue right behind the
                # drain squares so it never blocks (or is blocked by) the
                # sync queue's weight/store stream
                c0, c1 = ARB[k]
                nc.scalar.dma_start(ibs[k][:], stats[:, c0:c1])
                nc.gpsimd.collective_compute(
                    "AllReduce", OP.add,
                    replica_groups=[list(range(NCORES))],
                    ins=[ibs[k].opt()], outs=[obs[k].opt()])

            def fetch_red(k):
                c0, c1 = ARB[k]
                nc.sync.dma_start(red[:, c0:c1], obs[k][:])

            def finalize(m0, nt, sc, qc):
                sums = red[:, sc:sc + nt]
                sqs = red[:, qc:qc + nt]
                mean = small.tile([128, nt], F32, name=f"mean{m0}")
                nc.vector.tensor_single_scalar(
                    out=mean[:], in_=sums, scalar=1.0 / B, op=OP.mult)
                msq = small.tile([128, nt], F32, name=f"msq{m0}")
                nc.vector.tensor_single_scalar(
                    out=msq[:], in_=sqs, scalar=1.0 / B, op=OP.mult)
                var = small.tile([128, nt], F32, name=f"var{m0}")
                nc.vector.tensor_tensor(var[:], mean[:], mean[:], op=OP.mult)
                nc.vector.tensor_tensor(var[:], msq[:], var[:],
                                        op=OP.subtract)
                stdt = small.tile([128, nt], F32, name=f"std{m0}")
                nc.scalar.activation(stdt[:], var[:], AF.Sqrt, bias=epst[:])
                rstd = small.tile([128, nt], F32, name=f"rstd{m0}")
                nc.vector.reciprocal(out=rstd[:], in_=stdt[:])
                acols = ab[:, m0:m0 + nt]
                bcols = ab[:, NOT + m0:NOT + m0 + nt]
                nc.vector.tensor_tensor(acols, gmt[:, m0:m0 + nt], rstd[:],
                                        op=OP.mult)
                nc.vector.tensor_tensor(bcols, mean[:], acols, op=OP.mult)
                nc.vector.tensor_tensor(bcols, btt[:, m0:m0 + nt], bcols,
                                        op=OP.subtract)

            def normalize_store(g, split=False):
                # split=True: put the last tile's affine on ACT (Identity with
                # per-partition scale/bias) so the post-AllReduce tail chain
                # runs on two engines
                m0, nt = GROUPS[g]
                for ml in range(nt):
                    m = m0 + ml
                    if split and ml == nt - 1:
                        nc.scalar.activation(
                            yttiles[g][:, ml, :], ytiles[g][:, ml, :],
                            AF.Identity, bias=ab[:, NOT + m:NOT + m + 1],
                            scale=ab[:, m:m + 1])
                    else:
                        nc.vector.tensor_scalar(
                            out=yttiles[g][:, ml, :], in0=ytiles[g][:, ml, :],
                            scalar1=ab[:, m:m + 1],
                            scalar2=ab[:, NOT + m:NOT + m + 1],
                            op0=OP.mult, op1=OP.add)
                nc.sync.dma_start(yT_t[:, m0:m0 + nt, :], yttiles[g][:])

            # MM stream: g0u, g1u, g0w, g1w then g2, g3, g4 full
            mm_phase(0, 0)
            mm_phase(1, 0)
            mm_phase(0, 1)
            drain(0)
            mm_phase(1, 1)
            drain(1)
            allreduce("a")
            mm_phase(2, 0)
            mm_phase(2, 1)
            drain(2)
            allreduce("b")
            fetch_red("a")
            finalize(0, 4, 0, 4)
            finalize(4, 4, 8, 12)
            normalize_store(0)
            normalize_store(1)
            mm_phase(3, 0)
            mm_phase(3, 1)
            drain(3)
            mm_phase(4, 0)
            mm_phase(4, 1)
            drain(4)
            allreduce("c")
            fetch_red("b")
            finalize(8, 4, 16, 20)
            normalize_store(2)
            fetch_red("c")
            finalize(12, 4, 24, 28)
            normalize_store(3, split=True)
            normalize_store(4)

    nc.compile()
    return nc


def _get_nc():
    if "nc" not in _CACHE:
        _CACHE["nc"] = _build_nc()
    return _CACHE["nc"]


def _fold(v):
    """(1, D) or (D,) feature vector -> (128, NOT) column-per-o-tile layout."""
    return np.ascontiguousarray(np.asarray(v).reshape(NOT, 128).T).astype(
        np.float32)


def kernel(x, scale, translate, wave_weight, base_weight, gamma, beta):
    x = np.asarray(x, dtype=np.float32)
    scale = np.asarray(scale, dtype=np.float32).reshape(1, D)
    translate = np.asarray(translate, dtype=np.float32).reshape(1, D)
    wave_weight = np.asarray(wave_weight, dtype=np.float32)
    base_weight = np.asarray(base_weight, dtype=np.float32)
    gamma = np.asarray(gamma, dtype=np.float32).reshape(D)
    beta = np.asarray(beta, dtype=np.float32).reshape(D)

    sc = np.maximum(scale, 1e-3)                              # (1, D)
    u = (x - translate) / sc                                  # (B, D)

    # base_out = x @ Wb.T = u @ (diag(sc) Wb.T) + const_row; the const_row
    # shifts every batch row of a column equally, so BN's mean-subtraction
    # cancels it exactly and it is dropped.
    wbase = 0.3 * sc.reshape(D, 1) * base_weight.T            # (D, D) = [f,o]
    wwave = (math.pi ** -0.25) * wave_weight.T                # (D, D)
    wcat = np.ascontiguousarray(
        np.concatenate([wbase, wwave], axis=0).astype(np.float16))  # (2D, D)

    uT = np.ascontiguousarray(u.T.astype(np.float16))         # (D, B)

    cst = np.concatenate([_fold(gamma), _fold(beta)], axis=1)
    common = dict(wT=wcat, cst=np.ascontiguousarray(cst))
    in_maps = [
        dict(uT=np.ascontiguousarray(uT[:, c * BS:(c + 1) * BS]), **common)
        for c in range(NCORES)
    ]

    nc = _get_nc()
    res = run_bass_kernel_spmd(nc, in_maps, core_ids=list(range(NCORES)),
                               **_CACHE.pop("run_kwargs", {}))
    _CACHE["last_res"] = res
    yT = np.concatenate([res.results[c]["yT"] for c in range(NCORES)], axis=1)
    return np.ascontiguousarray(yT.T.astype(np.float32))
